# revision 13
# baseline (speedup 1.0000x reference)
"""Bahdanau-attention kernel for Trainium2 (8 NeuronCores, data-parallel over batch).

reference math:
  energy = relu(concat([hidden bcast T, enc], -1) @ W.T + b)   # [B,T,D]
  scores = energy @ v                                          # [B,T]
  out    = softmax(scores, axis=T)[:, None, :]                 # [B,1,T]

Per-core kernel (4 batch elems, 8192 bt rows):
  W = [W1 | W2] -> pre-energy[d, bt] = (enc @ W2.T).T + (hid @ W1.T + b)[d, b(bt)]
  hb = hid @ W1.T + b computed once on PE; folded into relu bias.
  enc tiles cast to bf16 (gpsimd cast-DMA), PE-transposed to [k, bt] layout,
  8x8 bf16 matmuls accumulate fp32 PSUM, ACT applies relu+bias -> bf16,
  PE v-dot matmul contracts d -> scores [1, 512], fp32 softmax over T at the end.
"""
import numpy as np
import ml_dtypes
import concourse.mybir as mybir
import concourse.tile as tile
import concourse.bacc as bacc
from concourse import bass_utils

P = 128
B, T, D = 32, 2048, 1024
N_CORES = 8
NB = B // N_CORES            # 4 local batch elems
BT = NB * T                  # 8192 local rows
BTT = 512                    # bt-tile (columns of energy^T)
N_BT = BT // BTT             # 16 bt-tiles
DT = D // P                  # 8 d-tiles (output dim of W)
KT = D // P                  # 8 k-tiles (contraction over enc features)
BF16, F32 = mybir.dt.bfloat16, mybir.dt.float32
RELU = mybir.ActivationFunctionType.Relu
EXP = mybir.ActivationFunctionType.Exp


def _build():
    nc = bacc.Bacc("TRN2", target_bir_lowering=False, debug=False)
    ENC = nc.dram_tensor("enc", [BT, D], F32, kind="ExternalInput").ap()
    HID = nc.dram_tensor("hid", [NB, D], F32, kind="ExternalInput").ap()
    WT = nc.dram_tensor("wt", [D, 2 * D], F32, kind="ExternalInput").ap()
    BIA = nc.dram_tensor("bia", [1, D], F32, kind="ExternalInput").ap()
    VV = nc.dram_tensor("vv", [1, D], F32, kind="ExternalInput").ap()
    IDN = nc.dram_tensor("idn", [P, P], BF16, kind="ExternalInput").ap()
    OUT = nc.dram_tensor("out", [NB, T], F32, kind="ExternalOutput").ap()

    with tile.TileContext(nc) as tc:
        with tc.tile_pool(name="persist", bufs=1) as pp:
            ident = pp.tile([P, P], BF16)
            nc.sync.dma_start(out=ident, in_=IDN)
            # persistent: transposed W halves, fused hidden/bias term, transposed v
            w1t = [pp.tile([P, D], BF16, name=f"w1t{j}") for j in range(KT)]
            w2t = [pp.tile([P, D], BF16, name=f"w2t{j}") for j in range(KT)]
            hb = pp.tile([P, DT * NB], F32)      # col di*NB+b = (hid@W1.T)[b, d] + bias[d]
            vt = pp.tile([P, DT], BF16)          # col di = v[di*128 : (di+1)*128]
            # batch elem bi lives on partition 32*bi (compute outputs need
            # 32-aligned partition bases)
            scores = pp.tile([P, T], F32)

            # ---- preamble: transpose W, build hb and vt ----
            with tc.tile_pool(name="pre_sb", bufs=1) as sp, \
                 tc.tile_pool(name="pre_ps", bufs=2, space="PSUM") as qp:
                wld = [sp.tile([P, 2 * D], BF16, name=f"wld{i}") for i in range(DT)]
                for i in range(DT):
                    nc.gpsimd.dma_start(out=wld[i], in_=WT[i * P:(i + 1) * P, :])
                hid_bf = sp.tile([NB, D], BF16)
                b_bf = sp.tile([1, D], BF16)
                v_bf = sp.tile([1, D], BF16)
                nc.gpsimd.dma_start(out=hid_bf, in_=HID)
                nc.gpsimd.dma_start(out=b_bf, in_=BIA)
                nc.gpsimd.dma_start(out=v_bf, in_=VV)
                ones = sp.tile([1, NB], BF16)
                nc.vector.memset(ones, 1.0)

                # W1T/W2T via DMA xbar transpose (bf16, SBUF->SBUF): keeps PE free
                for kj in range(2 * KT):
                    dst = w1t[kj] if kj < KT else w2t[kj - KT]
                    for i in range(DT):
                        nc.sync.dma_start(
                            out=dst[:, i * P:(i + 1) * P],
                            in_=wld[i][:, kj * P:(kj + 1) * P], transpose=True)

                # hT: [128, NB*KT], col kj*NB+b = hid[b, kj*128:...]
                ps_h = qp.tile([P, KT * NB], BF16, name="ps_h", bufs=1)
                for kj in range(KT):
                    nc.tensor.transpose(
                        ps_h[:, kj * NB:(kj + 1) * NB],
                        hid_bf[0:NB, kj * P:(kj + 1) * P], ident[0:NB, 0:NB])
                ht = sp.tile([P, KT * NB], BF16)
                nc.scalar.copy(ht, ps_h)

                # vT
                # single bf16 columns must land 4B-aligned in PSUM -> use even slots
                ps_v = qp.tile([P, 2 * DT], BF16, name="ps_v", bufs=1)
                for di in range(DT):
                    nc.tensor.transpose(
                        ps_v[:, 2 * di:2 * di + 1], v_bf[0:1, di * P:(di + 1) * P],
                        ident[0:1, 0:1])
                nc.scalar.copy(vt, ps_v.rearrange("p (d two) -> p d two", two=2)[:, :, 0])

                # hb[di] = sum_kj W1T[kj][:, di].T @ hT[:, kj] + b (via K=1 ones matmul)
                for di in range(DT):
                    ps_hb = qp.tile([P, NB], F32, name="ps_hb")
                    for kj in range(KT):
                        nc.tensor.matmul(
                            ps_hb, w1t[kj][:, di * P:(di + 1) * P],
                            ht[:, kj * NB:(kj + 1) * NB],
                            start=(kj == 0), stop=False)
                    nc.tensor.matmul(
                        ps_hb, b_bf[0:1, di * P:(di + 1) * P], ones[0:1, 0:NB],
                        start=False, stop=True)
                    nc.scalar.copy(hb[:, di * NB:(di + 1) * NB], ps_hb)

            # ---- main loop over bt-tiles ----
            with tc.tile_pool(name="enc_sb", bufs=8) as ep, \
                 tc.tile_pool(name="enct_sb", bufs=16) as tp, \
                 tc.tile_pool(name="e_sb", bufs=3) as ebp, \
                 tc.tile_pool(name="ps_e", bufs=4, space="PSUM") as pep, \
                 tc.tile_pool(name="ps_s", bufs=2, space="PSUM") as psp:

                enct = {}

                def load_and_transpose(n):
                    """DMA 512 enc rows (cast to bf16), DMA-xbar-transpose to [k, bt]."""
                    enc_bf = []
                    for j in range(4):
                        t_ = ep.tile([P, D], BF16, tag="enc", name=f"enc{n}_{j}")
                        r0 = n * BTT + j * P
                        nc.gpsimd.dma_start(out=t_, in_=ENC[r0:r0 + P, :])
                        enc_bf.append(t_)
                    tiles = []
                    for kj in range(KT):
                        t_ = tp.tile([P, BTT], BF16, tag="enct", name=f"enct{n}_{kj}")
                        for j in range(4):
                            nc.sync.dma_start(
                                out=t_[:, j * P:(j + 1) * P],
                                in_=enc_bf[j][:, kj * P:(kj + 1) * P], transpose=True)
                        tiles.append(t_)
                    enct[n] = tiles

                with tc.tile_pool(name="sm", bufs=1) as smp:
                    def softmax_row(bi):
                        """softmax over T for batch elem bi (scores row 32*bi)."""
                        row = scores[32 * bi:32 * bi + 1, :]
                        mx = smp.tile([1, 1], F32, tag="mx", name=f"mx{bi}", bufs=NB)
                        nc.vector.reduce_max(mx, row, axis=mybir.AxisListType.X)
                        nmx = smp.tile([1, 1], F32, tag="nmx", name=f"nmx{bi}", bufs=NB)
                        nc.vector.tensor_scalar_mul(nmx, mx, -1.0)
                        ex = smp.tile([1, T], F32, tag="ex", name=f"ex{bi}", bufs=2)
                        ssum = smp.tile([1, 1], F32, tag="ssum", name=f"ssum{bi}",
                                        bufs=NB)
                        nc.scalar.activation(ex, row, EXP, bias=nmx[:, 0:1],
                                             scale=1.0, accum_out=ssum)
                        rinv = smp.tile([1, 1], F32, tag="rinv", name=f"rinv{bi}",
                                        bufs=NB)
                        nc.vector.reciprocal(rinv, ssum)
                        o_sb = smp.tile([1, T], F32, tag="osb", name=f"osb{bi}",
                                        bufs=2)
                        nc.vector.tensor_scalar_mul(o_sb, ex, rinv[:, 0:1])
                        nc.sync.dma_start(out=OUT[bi:bi + 1, :], in_=o_sb)

                    load_and_transpose(0)
                    for n in range(N_BT):
                        bi = n // (T // BTT)
                        toff = (n % (T // BTT)) * BTT
                        if n + 1 < N_BT:
                            load_and_transpose(n + 1)
                        tiles = enct.pop(n)
                        ps_s = psp.tile([1, BTT], F32, tag="s", name=f"ps_s{n}")
                        pend = None  # (e_bf, di) vdot issued after next MM group
                        for di in range(DT):
                            ps_e = pep.tile([P, BTT], F32, tag="e",
                                            name=f"ps_e{n}_{di}")
                            for kj in range(KT):
                                nc.tensor.matmul(
                                    ps_e, w2t[kj][:, di * P:(di + 1) * P], tiles[kj],
                                    start=(kj == 0), stop=(kj == KT - 1))
                            if pend is not None:
                                eb, pdi = pend
                                nc.tensor.matmul(ps_s, vt[:, pdi:pdi + 1], eb,
                                                 start=(pdi == 0), stop=False)
                            e_bf = ebp.tile([P, BTT], BF16, tag="eb",
                                            name=f"e{n}_{di}")
                            nc.scalar.activation(
                                e_bf, ps_e, RELU,
                                bias=hb[:, di * NB + bi:di * NB + bi + 1],
                                scale=1.0)
                            pend = (e_bf, di)
                        eb, pdi = pend
                        nc.tensor.matmul(ps_s, vt[:, pdi:pdi + 1], eb,
                                         start=False, stop=True)
                        nc.vector.tensor_copy(
                            scores[32 * bi:32 * bi + 1, toff:toff + BTT], ps_s)
                        if n % (T // BTT) == (T // BTT) - 1:
                            softmax_row(bi)

    nc.compile()
    return nc


_NC_CACHE = []


def kernel(hidden, encoder_outputs, W, b, v):
    hidden = np.asarray(hidden, dtype=np.float32)
    enc = np.asarray(encoder_outputs, dtype=np.float32)
    W = np.asarray(W, dtype=np.float32)
    b = np.asarray(b, dtype=np.float32)
    v = np.asarray(v, dtype=np.float32)

    if not _NC_CACHE:
        _NC_CACHE.append(_build())
    nc = _NC_CACHE[0]

    ident = np.eye(P, dtype=np.float32).astype(ml_dtypes.bfloat16)
    b2 = b.reshape(1, D)
    v2 = v.reshape(1, D)
    in_maps = []
    for c in range(N_CORES):
        in_maps.append(dict(
            enc=enc[c * NB:(c + 1) * NB].reshape(BT, D),
            hid=hidden[c * NB:(c + 1) * NB],
            wt=W, bia=b2, vv=v2, idn=ident,
        ))
    res = bass_utils.run_bass_kernel_spmd(nc, in_maps, core_ids=list(range(N_CORES)))
    scores = np.concatenate([res.results[c]["out"] for c in range(N_CORES)], axis=0)
    return scores[:, None, :].astype(np.float32)


# revision 16
# speedup vs baseline: 2.8763x; 2.8763x over previous
"""Bahdanau-attention kernel for Trainium2 (8 NeuronCores, data-parallel over batch).

reference math:
  energy = relu(concat([hidden bcast T, enc], -1) @ W.T + b)   # [B,T,D]
  scores = energy @ v                                          # [B,T]
  out    = softmax(scores, axis=T)[:, None, :]                 # [B,1,T]

Per-core kernel (4 batch elems, 8192 bt rows):
  W = [W1 | W2] -> pre-energy[d, bt] = (enc @ W2.T).T + (hid @ W1.T + b)[d, b(bt)]
  hb = hid @ W1.T + b computed once on PE; folded into relu bias.
  enc tiles cast to bf16 (gpsimd cast-DMA), PE-transposed to [k, bt] layout,
  8x8 bf16 matmuls accumulate fp32 PSUM, ACT applies relu+bias -> bf16,
  PE v-dot matmul contracts d -> scores [1, 512], fp32 softmax over T at the end.
"""
import numpy as np
import ml_dtypes
import concourse.mybir as mybir
import concourse.tile as tile
import concourse.bacc as bacc
from concourse import bass_utils

P = 128
B, T, D = 32, 2048, 1024
N_CORES = 8
NB = B // N_CORES            # 4 local batch elems
BT = NB * T                  # 8192 local rows
BTT = 512                    # bt-tile (columns of energy^T)
N_BT = BT // BTT             # 16 bt-tiles
DT = D // P                  # 8 d-tiles (output dim of W)
KT = D // P                  # 8 k-tiles (contraction over enc features)
BF16, F32 = mybir.dt.bfloat16, mybir.dt.float32
RELU = mybir.ActivationFunctionType.Relu
EXP = mybir.ActivationFunctionType.Exp


def _build():
    nc = bacc.Bacc("TRN2", target_bir_lowering=False, debug=False)
    ENC = nc.dram_tensor("enc", [BT, D], F32, kind="ExternalInput").ap()
    HID = nc.dram_tensor("hid", [NB, D], F32, kind="ExternalInput").ap()
    WT = nc.dram_tensor("wt", [D, 2 * D], F32, kind="ExternalInput").ap()
    BIA = nc.dram_tensor("bia", [1, D], F32, kind="ExternalInput").ap()
    VV = nc.dram_tensor("vv", [1, D], F32, kind="ExternalInput").ap()
    IDN = nc.dram_tensor("idn", [P, P], BF16, kind="ExternalInput").ap()
    OUT = nc.dram_tensor("out", [NB, T], F32, kind="ExternalOutput").ap()

    with tile.TileContext(nc) as tc:
        with tc.tile_pool(name="persist", bufs=1) as pp:
            ident = pp.tile([P, P], BF16)
            nc.sync.dma_start(out=ident, in_=IDN)
            # persistent: transposed W halves, fused hidden/bias term, transposed v
            w1t = [pp.tile([P, D], BF16, name=f"w1t{j}") for j in range(KT)]
            w2t = [pp.tile([P, D], BF16, name=f"w2t{j}") for j in range(KT)]
            hb = pp.tile([P, DT * NB], F32)      # col di*NB+b = (hid@W1.T)[b, d] + bias[d]
            vt = pp.tile([P, DT], BF16)          # col di = v[di*128 : (di+1)*128]
            # batch elem bi lives on partition 32*bi (compute outputs need
            # 32-aligned partition bases)
            scores = pp.tile([P, T], F32)

            # ---- preamble: transpose W, build hb and vt ----
            with tc.tile_pool(name="pre_sb", bufs=1) as sp, \
                 tc.tile_pool(name="pre_ps", bufs=2, space="PSUM") as qp:
                wld = [sp.tile([P, 2 * D], BF16, name=f"wld{i}") for i in range(DT)]
                for i in range(DT):
                    nc.gpsimd.dma_start(out=wld[i], in_=WT[i * P:(i + 1) * P, :])
                hid_bf = sp.tile([NB, D], BF16)
                b_bf = sp.tile([1, D], BF16)
                v_bf = sp.tile([1, D], BF16)
                nc.gpsimd.dma_start(out=hid_bf, in_=HID)
                nc.gpsimd.dma_start(out=b_bf, in_=BIA)
                nc.gpsimd.dma_start(out=v_bf, in_=VV)
                ones = sp.tile([1, NB], BF16)
                nc.vector.memset(ones, 1.0)

                # W1T/W2T: 16 column-blocks of W, each transposed di-wise into one
                # psum bank (PE transpose; DMA xbar transpose measured 1.2us/block
                # serialized on Sync — far slower)
                for kj in range(2 * KT):
                    ps_w = qp.tile([P, D], BF16, name="ps_w")
                    for i in range(DT):
                        nc.tensor.transpose(
                            ps_w[:, i * P:(i + 1) * P],
                            wld[i][:, kj * P:(kj + 1) * P], ident)
                    dst = w1t[kj] if kj < KT else w2t[kj - KT]
                    nc.vector.tensor_copy(dst, ps_w)

                # hT: [128, NB*KT], col kj*NB+b = hid[b, kj*128:...]
                ps_h = qp.tile([P, KT * NB], BF16, name="ps_h", bufs=1)
                for kj in range(KT):
                    nc.tensor.transpose(
                        ps_h[:, kj * NB:(kj + 1) * NB],
                        hid_bf[0:NB, kj * P:(kj + 1) * P], ident[0:NB, 0:NB])
                ht = sp.tile([P, KT * NB], BF16)
                nc.scalar.copy(ht, ps_h)

                # vT
                # single bf16 columns must land 4B-aligned in PSUM -> use even slots
                ps_v = qp.tile([P, 2 * DT], BF16, name="ps_v", bufs=1)
                for di in range(DT):
                    nc.tensor.transpose(
                        ps_v[:, 2 * di:2 * di + 1], v_bf[0:1, di * P:(di + 1) * P],
                        ident[0:1, 0:1])
                nc.scalar.copy(vt, ps_v.rearrange("p (d two) -> p d two", two=2)[:, :, 0])

                # hb[di] = sum_kj W1T[kj][:, di].T @ hT[:, kj] + b (via K=1 ones matmul)
                for di in range(DT):
                    ps_hb = qp.tile([P, NB], F32, name="ps_hb")
                    for kj in range(KT):
                        nc.tensor.matmul(
                            ps_hb, w1t[kj][:, di * P:(di + 1) * P],
                            ht[:, kj * NB:(kj + 1) * NB],
                            start=(kj == 0), stop=False)
                    nc.tensor.matmul(
                        ps_hb, b_bf[0:1, di * P:(di + 1) * P], ones[0:1, 0:NB],
                        start=False, stop=True)
                    nc.scalar.copy(hb[:, di * NB:(di + 1) * NB], ps_hb)

            # ---- main loop over bt-tiles ----
            with tc.tile_pool(name="enc_sb", bufs=8) as ep, \
                 tc.tile_pool(name="enct_sb", bufs=16) as tp, \
                 tc.tile_pool(name="e_sb", bufs=3) as ebp, \
                 tc.tile_pool(name="ps_tr", bufs=3, space="PSUM") as trp, \
                 tc.tile_pool(name="ps_e", bufs=3, space="PSUM") as pep, \
                 tc.tile_pool(name="ps_s", bufs=2, space="PSUM") as psp:

                enct = {}

                def load_and_transpose(n):
                    """DMA 512 enc rows (cast to bf16), PE-transpose into [k, bt]."""
                    enc_bf = []
                    for j in range(4):
                        t_ = ep.tile([P, D], BF16, tag="enc", name=f"enc{n}_{j}")
                        r0 = n * BTT + j * P
                        nc.gpsimd.dma_start(out=t_, in_=ENC[r0:r0 + P, :])
                        enc_bf.append(t_)
                    tiles = []
                    for kj in range(KT):
                        ps_tr = trp.tile([P, BTT], BF16, tag="tr", name=f"ptr{n}_{kj}")
                        for j in range(4):
                            nc.tensor.transpose(
                                ps_tr[:, j * P:(j + 1) * P],
                                enc_bf[j][:, kj * P:(kj + 1) * P], ident)
                        t_ = tp.tile([P, BTT], BF16, tag="enct", name=f"enct{n}_{kj}")
                        nc.vector.tensor_copy(t_, ps_tr)
                        tiles.append(t_)
                    enct[n] = tiles

                with tc.tile_pool(name="sm", bufs=1) as smp:
                    def softmax_row(bi):
                        """softmax over T for batch elem bi (scores row 32*bi)."""
                        row = scores[32 * bi:32 * bi + 1, :]
                        mx = smp.tile([1, 1], F32, tag="mx", name=f"mx{bi}", bufs=NB)
                        nc.vector.reduce_max(mx, row, axis=mybir.AxisListType.X)
                        nmx = smp.tile([1, 1], F32, tag="nmx", name=f"nmx{bi}", bufs=NB)
                        nc.vector.tensor_scalar_mul(nmx, mx, -1.0)
                        ex = smp.tile([1, T], F32, tag="ex", name=f"ex{bi}", bufs=2)
                        ssum = smp.tile([1, 1], F32, tag="ssum", name=f"ssum{bi}",
                                        bufs=NB)
                        nc.scalar.activation(ex, row, EXP, bias=nmx[:, 0:1],
                                             scale=1.0, accum_out=ssum)
                        rinv = smp.tile([1, 1], F32, tag="rinv", name=f"rinv{bi}",
                                        bufs=NB)
                        nc.vector.reciprocal(rinv, ssum)
                        o_sb = smp.tile([1, T], F32, tag="osb", name=f"osb{bi}",
                                        bufs=2)
                        nc.vector.tensor_scalar_mul(o_sb, ex, rinv[:, 0:1])
                        nc.sync.dma_start(out=OUT[bi:bi + 1, :], in_=o_sb)

                    load_and_transpose(0)
                    for n in range(N_BT):
                        bi = n // (T // BTT)
                        toff = (n % (T // BTT)) * BTT
                        if n + 1 < N_BT:
                            load_and_transpose(n + 1)
                        tiles = enct.pop(n)
                        ps_s = psp.tile([1, BTT], F32, tag="s", name=f"ps_s{n}")
                        pend = None  # (e_bf, di) vdot issued after next MM group
                        for di in range(DT):
                            ps_e = pep.tile([P, BTT], F32, tag="e",
                                            name=f"ps_e{n}_{di}")
                            for kj in range(KT):
                                nc.tensor.matmul(
                                    ps_e, w2t[kj][:, di * P:(di + 1) * P], tiles[kj],
                                    start=(kj == 0), stop=(kj == KT - 1))
                            if pend is not None:
                                eb, pdi = pend
                                nc.tensor.matmul(ps_s, vt[:, pdi:pdi + 1], eb,
                                                 start=(pdi == 0), stop=False)
                            e_bf = ebp.tile([P, BTT], BF16, tag="eb",
                                            name=f"e{n}_{di}")
                            nc.scalar.activation(
                                e_bf, ps_e, RELU,
                                bias=hb[:, di * NB + bi:di * NB + bi + 1],
                                scale=1.0)
                            pend = (e_bf, di)
                        eb, pdi = pend
                        nc.tensor.matmul(ps_s, vt[:, pdi:pdi + 1], eb,
                                         start=False, stop=True)
                        nc.vector.tensor_copy(
                            scores[32 * bi:32 * bi + 1, toff:toff + BTT], ps_s)
                        if n % (T // BTT) == (T // BTT) - 1:
                            softmax_row(bi)

    nc.compile()
    return nc


_NC_CACHE = []


def kernel(hidden, encoder_outputs, W, b, v):
    hidden = np.asarray(hidden, dtype=np.float32)
    enc = np.asarray(encoder_outputs, dtype=np.float32)
    W = np.asarray(W, dtype=np.float32)
    b = np.asarray(b, dtype=np.float32)
    v = np.asarray(v, dtype=np.float32)

    if not _NC_CACHE:
        _NC_CACHE.append(_build())
    nc = _NC_CACHE[0]

    ident = np.eye(P, dtype=np.float32).astype(ml_dtypes.bfloat16)
    b2 = b.reshape(1, D)
    v2 = v.reshape(1, D)
    in_maps = []
    for c in range(N_CORES):
        in_maps.append(dict(
            enc=enc[c * NB:(c + 1) * NB].reshape(BT, D),
            hid=hidden[c * NB:(c + 1) * NB],
            wt=W, bia=b2, vv=v2, idn=ident,
        ))
    res = bass_utils.run_bass_kernel_spmd(nc, in_maps, core_ids=list(range(N_CORES)))
    scores = np.concatenate([res.results[c]["out"] for c in range(N_CORES)], axis=0)
    return scores[:, None, :].astype(np.float32)


# revision 21
# speedup vs baseline: 3.2004x; 1.1127x over previous
"""Bahdanau-attention kernel for Trainium2 (8 NeuronCores, data-parallel over batch).

reference math:
  energy = relu(concat([hidden bcast T, enc], -1) @ W.T + b)   # [B,T,D]
  scores = energy @ v                                          # [B,T]
  out    = softmax(scores, axis=T)[:, None, :]                 # [B,1,T]

Per-core kernel (4 batch elems, 8192 bt rows):
  W = [W1 | W2] -> pre-energy[d, bt] = (enc @ W2.T).T + (hid @ W1.T + b)[d, b(bt)]
  hb = hid @ W1.T + b computed once on PE; folded into the relu bias.
  enc tiles cast to bf16 (gpsimd cast-DMA), PE-transposed to [k, bt] layout,
  8x8 bf16 matmuls accumulate fp32 PSUM, ACT applies relu+bias -> bf16,
  v-dot contracts d via 4-wide col-group-packed PE matmuls (tile_position),
  cross-position DVE adds, fp32 softmax over T per batch elem.
"""
import numpy as np
import ml_dtypes
import concourse.mybir as mybir
import concourse.tile as tile
import concourse.bacc as bacc
from concourse import bass_utils

P = 128
B, T, D = 32, 2048, 1024
N_CORES = 8
NB = B // N_CORES            # 4 local batch elems
BT = NB * T                  # 8192 local rows
BTT = 512                    # bt-tile (columns of energy^T)
N_BT = BT // BTT             # 16 bt-tiles
DT = D // P                  # 8 d-tiles (output dim of W)
KT = D // P                  # 8 k-tiles (contraction over enc features)
BF16, F32 = mybir.dt.bfloat16, mybir.dt.float32
RELU = mybir.ActivationFunctionType.Relu
EXP = mybir.ActivationFunctionType.Exp


def _build():
    nc = bacc.Bacc("TRN2", target_bir_lowering=False, debug=False)
    ENC = nc.dram_tensor("enc", [BT, D], F32, kind="ExternalInput").ap()
    HID = nc.dram_tensor("hid", [NB, D], F32, kind="ExternalInput").ap()
    WT = nc.dram_tensor("wt", [D, 2 * D], F32, kind="ExternalInput").ap()
    BIA = nc.dram_tensor("bia", [1, D], F32, kind="ExternalInput").ap()
    VV = nc.dram_tensor("vv", [1, D], F32, kind="ExternalInput").ap()
    IDN = nc.dram_tensor("idn", [P, P], BF16, kind="ExternalInput").ap()
    OUT = nc.dram_tensor("out", [NB, T], F32, kind="ExternalOutput").ap()

    with tile.TileContext(nc) as tc, \
         tc.tile_pool(name="persist", bufs=1) as pp, \
         tc.tile_pool(name="pre_sb", bufs=1) as sp, \
         tc.tile_pool(name="enc_sb", bufs=12) as ep, \
         tc.tile_pool(name="enct_sb", bufs=24) as tp, \
         tc.tile_pool(name="e_sb", bufs=12) as ebp, \
         tc.tile_pool(name="ps_tr", bufs=3, space="PSUM") as trp, \
         tc.tile_pool(name="ps_e", bufs=3, space="PSUM") as pep, \
         tc.tile_pool(name="ps_s", bufs=2, space="PSUM") as psp, \
         tc.tile_pool(name="sm", bufs=1) as smp:

        ident = pp.tile([P, P], BF16)
        nc.sync.dma_start(out=ident, in_=IDN)
        # persistent: transposed W halves, fused hidden/bias term, transposed v
        w1t = [pp.tile([P, D], BF16, name=f"w1t{j}") for j in range(KT)]
        w2t = [pp.tile([P, D], BF16, name=f"w2t{j}") for j in range(KT)]
        hb = pp.tile([P, DT * NB], F32)  # col di*NB+b = (hid@W1.T)[b, d] + bias[d]
        vt = pp.tile([P, DT], BF16)      # col di = v[di*128 : (di+1)*128]
        # batch elem bi lives on partition 32*bi (compute outputs need
        # 32-aligned partition bases)
        scores = pp.tile([P, T], F32)

        enct = {}

        def load_tile(n):
            """gpsimd cast-DMA 512 enc rows fp32 -> 4 bf16 [128, 1024] tiles."""
            enc_bf = []
            for j in range(4):
                t_ = ep.tile([P, D], BF16, tag="enc", name=f"enc{n}_{j}")
                r0 = n * BTT + j * P
                nc.gpsimd.dma_start(out=t_, in_=ENC[r0:r0 + P, :])
                enc_bf.append(t_)
            return enc_bf

        def transpose_tile(n, enc_bf):
            """PE-transpose a 512-row block into 8 [k=128, bt=512] tiles."""
            tiles = []
            for kj in range(KT):
                ps_tr = trp.tile([P, BTT], BF16, tag="tr", name=f"ptr{n}_{kj}")
                for j in range(4):
                    nc.tensor.transpose(
                        ps_tr[:, j * P:(j + 1) * P],
                        enc_bf[j][:, kj * P:(kj + 1) * P], ident)
                t_ = tp.tile([P, BTT], BF16, tag="enct", name=f"enct{n}_{kj}")
                nc.vector.tensor_copy(t_, ps_tr)
                tiles.append(t_)
            enct[n] = tiles

        # ---- loads first: enc tiles 0/1 and the W2 half feed the PE earliest ----
        enc0 = load_tile(0)
        enc1 = load_tile(1)
        wld = [sp.tile([P, D], BF16, name=f"wld{i}", tag="wld", bufs=8)
               for i in range(2 * DT)]
        for i in range(DT):   # W2 half: columns D:2D
            nc.gpsimd.dma_start(out=wld[i], in_=WT[i * P:(i + 1) * P, D:2 * D])
        hid_bf = sp.tile([NB, D], BF16)
        b_bf = sp.tile([1, D], BF16)
        v_bf = sp.tile([1, D], BF16)
        nc.gpsimd.dma_start(out=hid_bf, in_=HID)
        nc.gpsimd.dma_start(out=b_bf, in_=BIA)
        nc.gpsimd.dma_start(out=v_bf, in_=VV)
        ones = sp.tile([1, NB], BF16)
        nc.vector.memset(ones, 1.0)

        # ---- early PE work: enc transposes for tiles 0/1 ----
        encraw = {2: load_tile(2)}
        transpose_tile(0, enc0)
        transpose_tile(1, enc1)

        # ---- W2T (needed by the first matmul group) ----
        def w_transpose(dst_list, base):
            for kj in range(KT):
                ps_w = trp.tile([P, D], BF16, tag="tr", name=f"psw{base}_{kj}")
                for i in range(DT):
                    nc.tensor.transpose(
                        ps_w[:, i * P:(i + 1) * P],
                        wld[base + i][:, kj * P:(kj + 1) * P], ident)
                nc.vector.tensor_copy(dst_list[kj], ps_w)

        w_transpose(w2t, 0)

        # ---- W1 half, hT, vT, hb (all must precede the first relu) ----
        for i in range(DT):   # W1 half: columns 0:D
            nc.gpsimd.dma_start(out=wld[DT + i], in_=WT[i * P:(i + 1) * P, 0:D])
        w_transpose(w1t, DT)

        # hT: [128, KT*NB], col kj*NB+b = hid[b, kj*128:...]
        ps_h = pep.tile([P, KT * NB], BF16, tag="e", name="ps_h")
        for kj in range(KT):
            nc.tensor.transpose(
                ps_h[:, kj * NB:(kj + 1) * NB],
                hid_bf[0:NB, kj * P:(kj + 1) * P], ident[0:NB, 0:NB])
        ht = sp.tile([P, KT * NB], BF16)
        nc.scalar.copy(ht, ps_h)

        # vT (single bf16 psum columns must land 4B-aligned -> even slots)
        ps_v = pep.tile([P, 2 * DT], BF16, tag="e", name="ps_v")
        for di in range(DT):
            nc.tensor.transpose(
                ps_v[:, 2 * di:2 * di + 1], v_bf[0:1, di * P:(di + 1) * P],
                ident[0:1, 0:1])
        nc.scalar.copy(vt, ps_v.rearrange("p (d two) -> p d two", two=2)[:, :, 0])

        # hb[di] = sum_kj W1T[kj][:, di].T @ hT[:, kj] + b (K=1 ones matmul)
        for di in range(DT):
            ps_hb = pep.tile([P, NB], F32, tag="e", name=f"ps_hb{di}")
            for kj in range(KT):
                nc.tensor.matmul(
                    ps_hb, w1t[kj][:, di * P:(di + 1) * P],
                    ht[:, kj * NB:(kj + 1) * NB],
                    start=(kj == 0), stop=False)
            nc.tensor.matmul(
                ps_hb, b_bf[0:1, di * P:(di + 1) * P], ones[0:1, 0:NB],
                start=False, stop=True)
            nc.scalar.copy(hb[:, di * NB:(di + 1) * NB], ps_hb)

        # ---- softmax over T for one batch elem (scores row 32*bi) ----
        def softmax_row(bi):
            row = scores[32 * bi:32 * bi + 1, :]
            mx = smp.tile([1, 1], F32, tag="mx", name=f"mx{bi}", bufs=NB)
            nc.vector.reduce_max(mx, row, axis=mybir.AxisListType.X)
            nmx = smp.tile([1, 1], F32, tag="nmx", name=f"nmx{bi}", bufs=NB)
            nc.vector.tensor_scalar_mul(nmx, mx, -1.0)
            ex = smp.tile([1, T], F32, tag="ex", name=f"ex{bi}", bufs=2)
            ssum = smp.tile([1, 1], F32, tag="ssum", name=f"ssum{bi}", bufs=NB)
            nc.scalar.activation(ex, row, EXP, bias=nmx[:, 0:1], scale=1.0,
                                 accum_out=ssum)
            rinv = smp.tile([1, 1], F32, tag="rinv", name=f"rinv{bi}", bufs=NB)
            nc.vector.reciprocal(rinv, ssum)
            o_sb = smp.tile([1, T], F32, tag="osb", name=f"osb{bi}", bufs=2)
            nc.vector.tensor_scalar_mul(o_sb, ex, rinv[:, 0:1])
            nc.sync.dma_start(out=OUT[bi:bi + 1, :], in_=o_sb)

        # ---- v-dot: 8 M=1 matmuls packed 4-wide into PE column groups ----
        def flush_vdots(pend):
            ps_s, e_list, bi, toff = pend
            for di in range(DT):
                jj = di % 4
                nc.tensor.matmul(
                    ps_s[32 * jj:32 * jj + 1, :], vt[:, di:di + 1], e_list[di],
                    start=(di < 4), stop=(di >= 4),
                    tile_position=(0, 32 * jj))
            # cross-position reduction (PSUM has 1 DVE read port -> stage via SBUF)
            sacc = smp.tile([1, BTT], F32, tag="sacc", name=f"sacc{toff}_{bi}",
                            bufs=2)
            nc.scalar.copy(sacc, ps_s[0:1, :])
            nc.vector.tensor_add(sacc, sacc, ps_s[32:33, :])
            nc.vector.tensor_add(sacc, sacc, ps_s[64:65, :])
            nc.vector.tensor_add(
                scores[32 * bi:32 * bi + 1, toff:toff + BTT],
                sacc, ps_s[96:97, :])
            if toff == T - BTT:
                softmax_row(bi)

        # ---- main loop over bt-tiles ----
        # pipeline: load n+3 (DMA), transpose n+2 (PE, data loaded last iter),
        # matmul n. Keeps one full tile period between a load and its use.
        pend = None
        for n in range(N_BT):
            bi = n // (T // BTT)
            toff = (n % (T // BTT)) * BTT
            if n + 3 < N_BT:
                encraw[n + 3] = load_tile(n + 3)
            if n + 2 < N_BT:
                transpose_tile(n + 2, encraw.pop(n + 2))
            tiles = enct.pop(n)
            ps_s = psp.tile([P, BTT], F32, tag="s", name=f"ps_s{n}")
            e_list = []
            for di in range(DT):
                ps_e = pep.tile([P, BTT], F32, tag="e", name=f"ps_e{n}_{di}")
                for kj in range(KT):
                    nc.tensor.matmul(
                        ps_e, w2t[kj][:, di * P:(di + 1) * P], tiles[kj],
                        start=(kj == 0), stop=(kj == KT - 1))
                if di == 1 and pend is not None:
                    flush_vdots(pend)
                    pend = None
                e_bf = ebp.tile([P, BTT], BF16, tag="eb", name=f"e{n}_{di}")
                nc.scalar.activation(
                    e_bf, ps_e, RELU,
                    bias=hb[:, di * NB + bi:di * NB + bi + 1], scale=1.0)
                e_list.append(e_bf)
            pend = (ps_s, e_list, bi, toff)
        flush_vdots(pend)

    nc.compile()
    return nc


_NC_CACHE = []


def kernel(hidden, encoder_outputs, W, b, v):
    hidden = np.asarray(hidden, dtype=np.float32)
    enc = np.asarray(encoder_outputs, dtype=np.float32)
    W = np.asarray(W, dtype=np.float32)
    b = np.asarray(b, dtype=np.float32)
    v = np.asarray(v, dtype=np.float32)

    if not _NC_CACHE:
        _NC_CACHE.append(_build())
    nc = _NC_CACHE[0]

    ident = np.eye(P, dtype=np.float32).astype(ml_dtypes.bfloat16)
    b2 = b.reshape(1, D)
    v2 = v.reshape(1, D)
    in_maps = []
    for c in range(N_CORES):
        in_maps.append(dict(
            enc=enc[c * NB:(c + 1) * NB].reshape(BT, D),
            hid=hidden[c * NB:(c + 1) * NB],
            wt=W, bia=b2, vv=v2, idn=ident,
        ))
    res = bass_utils.run_bass_kernel_spmd(nc, in_maps, core_ids=list(range(N_CORES)))
    scores = np.concatenate([res.results[c]["out"] for c in range(N_CORES)], axis=0)
    return scores[:, None, :].astype(np.float32)


# revision 22
# speedup vs baseline: 3.2483x; 1.0150x over previous
"""Bahdanau-attention kernel for Trainium2 (8 NeuronCores, data-parallel over batch).

reference math:
  energy = relu(concat([hidden bcast T, enc], -1) @ W.T + b)   # [B,T,D]
  scores = energy @ v                                          # [B,T]
  out    = softmax(scores, axis=T)[:, None, :]                 # [B,1,T]

Per-core kernel (4 batch elems, 8192 bt rows):
  W = [W1 | W2] -> pre-energy[d, bt] = (enc @ W2.T).T + (hid @ W1.T + b)[d, b(bt)]
  hb = hid @ W1.T + b computed once on PE; folded into the relu bias.
  enc tiles cast to bf16 (gpsimd cast-DMA), PE-transposed to [k, bt] layout,
  8x8 bf16 matmuls accumulate fp32 PSUM, ACT applies relu+bias -> bf16,
  v-dot contracts d via 4-wide col-group-packed PE matmuls (tile_position),
  cross-position DVE adds, fp32 softmax over T per batch elem.
"""
import numpy as np
import ml_dtypes
import concourse.mybir as mybir
import concourse.tile as tile
import concourse.bacc as bacc
from concourse import bass_utils

P = 128
B, T, D = 32, 2048, 1024
N_CORES = 8
NB = B // N_CORES            # 4 local batch elems
BT = NB * T                  # 8192 local rows
BTT = 512                    # bt-tile (columns of energy^T)
N_BT = BT // BTT             # 16 bt-tiles
DT = D // P                  # 8 d-tiles (output dim of W)
KT = D // P                  # 8 k-tiles (contraction over enc features)
BF16, F32 = mybir.dt.bfloat16, mybir.dt.float32
RELU = mybir.ActivationFunctionType.Relu
EXP = mybir.ActivationFunctionType.Exp


def _build():
    nc = bacc.Bacc("TRN2", target_bir_lowering=False, debug=False)
    ENC = nc.dram_tensor("enc", [BT, D], F32, kind="ExternalInput").ap()
    HID = nc.dram_tensor("hid", [NB, D], F32, kind="ExternalInput").ap()
    WT = nc.dram_tensor("wt", [D, 2 * D], F32, kind="ExternalInput").ap()
    BIA = nc.dram_tensor("bia", [1, D], F32, kind="ExternalInput").ap()
    VV = nc.dram_tensor("vv", [1, D], F32, kind="ExternalInput").ap()
    IDN = nc.dram_tensor("idn", [P, P], BF16, kind="ExternalInput").ap()
    OUT = nc.dram_tensor("out", [NB, T], F32, kind="ExternalOutput").ap()

    with tile.TileContext(nc) as tc, \
         tc.tile_pool(name="persist", bufs=1) as pp, \
         tc.tile_pool(name="pre_sb", bufs=1) as sp, \
         tc.tile_pool(name="enc_sb", bufs=12) as ep, \
         tc.tile_pool(name="enct_sb", bufs=24) as tp, \
         tc.tile_pool(name="e_sb", bufs=12) as ebp, \
         tc.tile_pool(name="ps_tr", bufs=3, space="PSUM") as trp, \
         tc.tile_pool(name="ps_e", bufs=3, space="PSUM") as pep, \
         tc.tile_pool(name="ps_s", bufs=2, space="PSUM") as psp, \
         tc.tile_pool(name="sm", bufs=1) as smp:

        ident = pp.tile([P, P], BF16)
        nc.sync.dma_start(out=ident, in_=IDN)
        # persistent: transposed W halves, fused hidden/bias term, transposed v
        w1t = [pp.tile([P, D], BF16, name=f"w1t{j}") for j in range(KT)]
        w2t = [pp.tile([P, D], BF16, name=f"w2t{j}") for j in range(KT)]
        hb = pp.tile([P, DT * NB], F32)  # col di*NB+b = (hid@W1.T)[b, d] + bias[d]
        vt = pp.tile([P, DT], BF16)      # col di = v[di*128 : (di+1)*128]
        # batch elem bi lives on partition 32*bi (compute outputs need
        # 32-aligned partition bases)
        scores = pp.tile([P, T], F32)

        enct = {}

        def load_tile(n):
            """gpsimd cast-DMA 512 enc rows fp32 -> 4 bf16 [128, 1024] tiles."""
            enc_bf = []
            for j in range(4):
                t_ = ep.tile([P, D], BF16, tag="enc", name=f"enc{n}_{j}")
                r0 = n * BTT + j * P
                nc.gpsimd.dma_start(out=t_, in_=ENC[r0:r0 + P, :])
                enc_bf.append(t_)
            return enc_bf

        def transpose_tile(n, enc_bf):
            """PE-transpose a 512-row block into 8 [k=128, bt=512] tiles."""
            tiles = []
            for kj in range(KT):
                ps_tr = trp.tile([P, BTT], BF16, tag="tr", name=f"ptr{n}_{kj}")
                for j in range(4):
                    nc.tensor.transpose(
                        ps_tr[:, j * P:(j + 1) * P],
                        enc_bf[j][:, kj * P:(kj + 1) * P], ident)
                t_ = tp.tile([P, BTT], BF16, tag="enct", name=f"enct{n}_{kj}")
                nc.vector.tensor_copy(t_, ps_tr)
                tiles.append(t_)
            enct[n] = tiles

        # ---- loads first: enc tiles 0/1 and the W2 half feed the PE earliest ----
        enc0 = load_tile(0)
        enc1 = load_tile(1)
        wld = [sp.tile([P, D], BF16, name=f"wld{i}", tag="wld", bufs=8)
               for i in range(2 * DT)]
        for i in range(DT):   # W2 half: columns D:2D
            nc.gpsimd.dma_start(out=wld[i], in_=WT[i * P:(i + 1) * P, D:2 * D])
        hid_bf = sp.tile([NB, D], BF16)
        b_bf = sp.tile([1, D], BF16)
        v_bf = sp.tile([1, D], BF16)
        nc.gpsimd.dma_start(out=hid_bf, in_=HID)
        nc.gpsimd.dma_start(out=b_bf, in_=BIA)
        nc.gpsimd.dma_start(out=v_bf, in_=VV)
        ones = sp.tile([1, NB], BF16)
        nc.vector.memset(ones, 1.0)

        # ---- early PE work: enc transposes for tiles 0/1 ----
        encraw = {2: load_tile(2)}
        transpose_tile(0, enc0)
        transpose_tile(1, enc1)

        # ---- W2T (needed by the first matmul group) ----
        # i-outer so each row-tile's transposes start as soon as ITS load lands
        # (kj-outer would stall the PE until all 8 W loads finish)
        def w_transpose(dst_list, base):
            for i in range(DT):
                ps_w = trp.tile([P, D], BF16, tag="tr", name=f"psw{base}_{i}")
                for kj in range(KT):
                    nc.tensor.transpose(
                        ps_w[:, kj * P:(kj + 1) * P],
                        wld[base + i][:, kj * P:(kj + 1) * P], ident)
                for kj in range(KT):
                    nc.vector.tensor_copy(
                        dst_list[kj][:, i * P:(i + 1) * P],
                        ps_w[:, kj * P:(kj + 1) * P])

        w_transpose(w2t, 0)

        # ---- W1 half, hT, vT, hb (all must precede the first relu) ----
        for i in range(DT):   # W1 half: columns 0:D
            nc.gpsimd.dma_start(out=wld[DT + i], in_=WT[i * P:(i + 1) * P, 0:D])
        w_transpose(w1t, DT)

        # hT: [128, KT*NB], col kj*NB+b = hid[b, kj*128:...]
        ps_h = pep.tile([P, KT * NB], BF16, tag="e", name="ps_h")
        for kj in range(KT):
            nc.tensor.transpose(
                ps_h[:, kj * NB:(kj + 1) * NB],
                hid_bf[0:NB, kj * P:(kj + 1) * P], ident[0:NB, 0:NB])
        ht = sp.tile([P, KT * NB], BF16)
        nc.scalar.copy(ht, ps_h)

        # vT (single bf16 psum columns must land 4B-aligned -> even slots)
        ps_v = pep.tile([P, 2 * DT], BF16, tag="e", name="ps_v")
        for di in range(DT):
            nc.tensor.transpose(
                ps_v[:, 2 * di:2 * di + 1], v_bf[0:1, di * P:(di + 1) * P],
                ident[0:1, 0:1])
        nc.scalar.copy(vt, ps_v.rearrange("p (d two) -> p d two", two=2)[:, :, 0])

        # hb[di] = sum_kj W1T[kj][:, di].T @ hT[:, kj] + b (K=1 ones matmul)
        for di in range(DT):
            ps_hb = pep.tile([P, NB], F32, tag="e", name=f"ps_hb{di}")
            for kj in range(KT):
                nc.tensor.matmul(
                    ps_hb, w1t[kj][:, di * P:(di + 1) * P],
                    ht[:, kj * NB:(kj + 1) * NB],
                    start=(kj == 0), stop=False)
            nc.tensor.matmul(
                ps_hb, b_bf[0:1, di * P:(di + 1) * P], ones[0:1, 0:NB],
                start=False, stop=True)
            nc.scalar.copy(hb[:, di * NB:(di + 1) * NB], ps_hb)

        # ---- softmax over T for one batch elem (scores row 32*bi) ----
        def softmax_row(bi):
            row = scores[32 * bi:32 * bi + 1, :]
            mx = smp.tile([1, 1], F32, tag="mx", name=f"mx{bi}", bufs=NB)
            nc.vector.reduce_max(mx, row, axis=mybir.AxisListType.X)
            nmx = smp.tile([1, 1], F32, tag="nmx", name=f"nmx{bi}", bufs=NB)
            nc.vector.tensor_scalar_mul(nmx, mx, -1.0)
            ex = smp.tile([1, T], F32, tag="ex", name=f"ex{bi}", bufs=2)
            ssum = smp.tile([1, 1], F32, tag="ssum", name=f"ssum{bi}", bufs=NB)
            nc.scalar.activation(ex, row, EXP, bias=nmx[:, 0:1], scale=1.0,
                                 accum_out=ssum)
            rinv = smp.tile([1, 1], F32, tag="rinv", name=f"rinv{bi}", bufs=NB)
            nc.vector.reciprocal(rinv, ssum)
            o_sb = smp.tile([1, T], F32, tag="osb", name=f"osb{bi}", bufs=2)
            nc.vector.tensor_scalar_mul(o_sb, ex, rinv[:, 0:1])
            nc.sync.dma_start(out=OUT[bi:bi + 1, :], in_=o_sb)

        # ---- v-dot: 8 M=1 matmuls packed 4-wide into PE column groups ----
        def flush_vdots(pend):
            ps_s, e_list, bi, toff = pend
            for di in range(DT):
                jj = di % 4
                nc.tensor.matmul(
                    ps_s[32 * jj:32 * jj + 1, :], vt[:, di:di + 1], e_list[di],
                    start=(di < 4), stop=(di >= 4),
                    tile_position=(0, 32 * jj))
            # cross-position reduction (PSUM has 1 DVE read port -> stage via SBUF)
            sacc = smp.tile([1, BTT], F32, tag="sacc", name=f"sacc{toff}_{bi}",
                            bufs=2)
            nc.scalar.copy(sacc, ps_s[0:1, :])
            nc.vector.tensor_add(sacc, sacc, ps_s[32:33, :])
            nc.vector.tensor_add(sacc, sacc, ps_s[64:65, :])
            nc.vector.tensor_add(
                scores[32 * bi:32 * bi + 1, toff:toff + BTT],
                sacc, ps_s[96:97, :])
            if toff == T - BTT:
                softmax_row(bi)

        # ---- main loop over bt-tiles ----
        # pipeline: load n+3 (DMA), transpose n+2 (PE, data loaded last iter),
        # matmul n. Keeps one full tile period between a load and its use.
        pend = None
        for n in range(N_BT):
            bi = n // (T // BTT)
            toff = (n % (T // BTT)) * BTT
            if n + 3 < N_BT:
                encraw[n + 3] = load_tile(n + 3)
            if n + 2 < N_BT:
                transpose_tile(n + 2, encraw.pop(n + 2))
            tiles = enct.pop(n)
            ps_s = psp.tile([P, BTT], F32, tag="s", name=f"ps_s{n}")
            e_list = []
            for di in range(DT):
                ps_e = pep.tile([P, BTT], F32, tag="e", name=f"ps_e{n}_{di}")
                for kj in range(KT):
                    nc.tensor.matmul(
                        ps_e, w2t[kj][:, di * P:(di + 1) * P], tiles[kj],
                        start=(kj == 0), stop=(kj == KT - 1))
                if di == 1 and pend is not None:
                    flush_vdots(pend)
                    pend = None
                e_bf = ebp.tile([P, BTT], BF16, tag="eb", name=f"e{n}_{di}")
                nc.scalar.activation(
                    e_bf, ps_e, RELU,
                    bias=hb[:, di * NB + bi:di * NB + bi + 1], scale=1.0)
                e_list.append(e_bf)
            pend = (ps_s, e_list, bi, toff)
        flush_vdots(pend)

    nc.compile()
    return nc


_NC_CACHE = []


def kernel(hidden, encoder_outputs, W, b, v):
    hidden = np.asarray(hidden, dtype=np.float32)
    enc = np.asarray(encoder_outputs, dtype=np.float32)
    W = np.asarray(W, dtype=np.float32)
    b = np.asarray(b, dtype=np.float32)
    v = np.asarray(v, dtype=np.float32)

    if not _NC_CACHE:
        _NC_CACHE.append(_build())
    nc = _NC_CACHE[0]

    ident = np.eye(P, dtype=np.float32).astype(ml_dtypes.bfloat16)
    b2 = b.reshape(1, D)
    v2 = v.reshape(1, D)
    in_maps = []
    for c in range(N_CORES):
        in_maps.append(dict(
            enc=enc[c * NB:(c + 1) * NB].reshape(BT, D),
            hid=hidden[c * NB:(c + 1) * NB],
            wt=W, bia=b2, vv=v2, idn=ident,
        ))
    res = bass_utils.run_bass_kernel_spmd(nc, in_maps, core_ids=list(range(N_CORES)))
    scores = np.concatenate([res.results[c]["out"] for c in range(N_CORES)], axis=0)
    return scores[:, None, :].astype(np.float32)
